# revision 19
# baseline (speedup 1.0000x reference)
"""Trainium2 Bass kernel v5 for bucketed causal linear self-attention.

Model (B=4, T=4096, DIM=1024, H=16 heads, E=64, BUCKET=64):
  q,k,v = x@Wq, x@Wk, x@Wv ; q softmaxed over head-dim, k -> elu(k)+1
  per-bucket context C_u = cumsum_u(k_bu^T v_bu), normalized by cumsum of
  key-sums, shifted one bucket; attn_bu = q_bu @ C_{u-1}; out = attn@Wo + bo.

Sharding over 8 cores: core c -> batch c//2, head-group c%2 (8 heads = 512
feats). Host transposes x (so no on-device DMA transposes), sums the two
partial outputs per batch and adds bo.

v5 vs v2:
  - steady-state slot schedule: walk c's bucket slots carry chunk c's OWN
    kv tt1-3 (b0-2), chunk c+1's q pieces (b3-6, + sm fin), and chunk
    c+1's kv tt0 (b7). This shrinks chunk 0's serial pre-walk phase from
    q+all-kv to q+kv(tt0) and gives the last walk real filler.
  - prologue DMAs ordered by first use (wq+xT0 interleaved, then wk, wv,
    wo halves, orp) across the three DMA-capable queues.
  - chunk-0 q-projection runs kt-outer over 4 concurrent PSUM banks so
    matmuls start as each (wq kt, xT kt) pair lands.
  - proj-tile PSUM allocs happen at slot b3 (after the q2 multiplies
    consumed the broadcast banks) to avoid bank-recycle stalls.
  - q2 = E_t * rp with rp read straight from PSUM by the DVE.
  - cbf snapshot is one DVE op per bucket.
  - last chunk interleaves out-proj half-pieces at every bucket; final
    stores split per half to drain earlier.
"""

import sys
import numpy as np
import ml_dtypes

sys.path.insert(0, "/opt/trn_rl_repo")

B, T, DIM, H, BUCKET = 4, 4096, 1024, 16, 64
E = 64           # head dim
HC = 8           # heads per core
F = HC * E       # per-core feature width = 512
CH = 512         # tokens per chunk
UC = CH // BUCKET  # buckets per chunk = 8
PAIRS = HC // 2  # head pairs = 4
KT = DIM // 128  # contraction tiles = 8
NCH = T // CH    # chunks = 8

_NC_CACHE = {}


def build_nc(n_chunks=NCH):
    import concourse.bass as bass
    import concourse.mybir as mybir
    from concourse import bacc
    from concourse.tile import TileContext

    BF16 = mybir.dt.bfloat16
    F32 = mybir.dt.float32
    AF = mybir.ActivationFunctionType
    OP = mybir.AluOpType

    Tt = n_chunks * CH

    nc = bacc.Bacc("TRN2", target_bir_lowering=False, debug=False, num_devices=8)
    # weights arrive host-prearranged partition-major so loads are contiguous
    xt = nc.dram_tensor("xt", [128, KT, Tt], BF16, kind="ExternalInput").ap()
    wq = nc.dram_tensor("wq", [128, KT * F], BF16, kind="ExternalInput").ap()
    wk = nc.dram_tensor("wk", [128, KT * F], BF16, kind="ExternalInput").ap()
    wv = nc.dram_tensor("wv", [128, KT * F], BF16, kind="ExternalInput").ap()
    wo = nc.dram_tensor("wo", [128, PAIRS * DIM], BF16, kind="ExternalInput").ap()
    orp = nc.dram_tensor("orp", [HC, PAIRS * 128], BF16, kind="ExternalInput").ap()
    out = nc.dram_tensor("out", [Tt, DIM], BF16, kind="ExternalOutput").ap()

    xt_r = xt

    with TileContext(nc) as tc:
        with tc.tile_pool(name="const", bufs=1) as constp, \
             tc.tile_pool(name="xt", bufs=2) as xtp, \
             tc.tile_pool(name="act", bufs=2) as actp, \
             tc.tile_pool(name="tmp", bufs=3) as tmpp, \
             tc.tile_pool(name="small", bufs=6) as smallp, \
             tc.tile_pool(name="cbfp", bufs=6) as cbfp, \
             tc.tile_pool(name="outp", bufs=3) as outp, \
             tc.tile_pool(name="ps_proj", bufs=5, space="PSUM") as psP, \
             tc.tile_pool(name="ps_atn", bufs=2, space="PSUM") as psA, \
             tc.tile_pool(name="ps_c", bufs=1, space="PSUM") as psC:

            # ---- resident weights + chunk-0 xT: issue in order of first
            # use so the 16 HW DMA engines serve critical bytes first.
            wq_sb = constp.tile([128, KT, F], BF16, tag="wq")
            wk_sb = constp.tile([128, KT, F], BF16, tag="wk")
            wv_sb = constp.tile([128, KT, F], BF16, tag="wv")
            wo_sb = constp.tile([128, PAIRS, DIM], BF16, tag="wo")
            wq_r = wq.rearrange("p (kt f) -> p kt f", f=F)
            wk_r = wk.rearrange("p (kt f) -> p kt f", f=F)
            wv_r = wv.rearrange("p (kt f) -> p kt f", f=F)
            wo_r = wo.rearrange("p (ft n) -> p ft n", n=DIM)

            xT0 = xtp.tile([128, KT, CH], BF16, tag="xT")
            for g in range(4):
                qeng = nc.scalar if g % 2 == 0 else nc.gpsimd
                qeng.dma_start(out=wq_sb[:, 2 * g:2 * g + 2, :],
                               in_=wq_r[:, 2 * g:2 * g + 2, :])
                nc.sync.dma_start(out=xT0[:, 2 * g:2 * g + 2, :],
                                  in_=xt_r[:, 2 * g:2 * g + 2, 0:CH])
            for g in range(4):
                eng = nc.scalar if g % 2 == 0 else nc.gpsimd
                eng.dma_start(out=wk_sb[:, 2 * g:2 * g + 2, :],
                              in_=wk_r[:, 2 * g:2 * g + 2, :])
            for g in range(4):
                eng = nc.scalar if g % 2 == 0 else nc.gpsimd
                eng.dma_start(out=wv_sb[:, 2 * g:2 * g + 2, :],
                              in_=wv_r[:, 2 * g:2 * g + 2, :])
            # ones_repl[:, p, :]: [8,128] stationary; row 2p+hh is 1 on cols hh*64..
            ones_repl = constp.tile([HC, PAIRS, 128], BF16, tag="ones_repl")
            nc.sync.dma_start(
                out=ones_repl[:],
                in_=orp.rearrange("h (p c) -> h p c", c=128))
            for p in range(PAIRS):
                eng = nc.scalar if p % 2 == 0 else nc.gpsimd
                eng.dma_start(out=wo_sb[:, p, :], in_=wo_r[:, p, :])

            # zero tiles first: the C-init matmul heads the tensor FIFO and
            # only needs these two memsets
            zcbf = constp.tile([128, PAIRS, E + 1], BF16, tag="zcbf")
            nc.vector.memset(zcbf[:], 0.0)
            ztile = constp.tile([64, 128], BF16, tag="ztile")
            nc.vector.memset(ztile[:], 0.0)
            # ones_sel[:, p, :]: [128,8] stationary; col 2p+hh is 1 on rows hh*64..
            ones_sel = constp.tile([128, PAIRS, HC], BF16, tag="ones_sel")
            nc.vector.memset(ones_sel[:], 0.0)
            for p in range(PAIRS):
                for hh in range(2):
                    nc.vector.memset(
                        ones_sel[hh * 64:(hh + 1) * 64, p, 2 * p + hh:2 * p + hh + 1], 1.0)
            # zcbd is the block-diagonal [128,128]-per-pair zero context for
            # the very first bucket (blindspot)
            zcbd = constp.tile([128, PAIRS, 128], BF16, tag="zcbd")
            nc.vector.memset(zcbd[:], 0.0)
            # pre-zero all cbf pool buffers once: walk uses only rewrite the
            # two diagonal blocks, so the off-diagonal zeros persist.
            for zi in range(6):
                zb = cbfp.tile([128, PAIRS, 128], BF16, tag="cbf", name=f"cbz{zi}")
                nc.vector.memset(zb[:], 0.0)

            # HAM warm-up: dep-free matmuls on the zero tiles keep the PE
            # busy while the first weight/x DMAs are in flight, so the real
            # projection matmuls start at 2.4 GHz instead of 1.2.
            warm = psA.tile([128, PAIRS, BUCKET], F32, tag="atn", name="warm")
            for _ in range(8):
                nc.tensor.matmul(
                    warm[0:64, :, :].rearrange("p a b -> p (a b)")[:, 0:192],
                    ztile[:, 0:64],
                    zcbf[0:64, :, :].rearrange("p a b -> p (a b)")[:, 0:192],
                    start=True, stop=True)

            # running context+ksum per head pair, PSUM-resident: [2*64 d, p, 64 e + 1 ks]
            C = psC.tile([128, PAIRS, E + 1], F32, tag="C")
            # Zero-init the whole C region with ONE start=True matmul: the
            # bank-wide has_written clear must happen exactly once, before
            # any S write, and the WAW overlap with every later quadrant
            # write pins this matmul first in the schedule.
            nc.tensor.matmul(C[:].rearrange("p a b -> p (a b)"), ztile[:],
                             zcbf[0:64, :, :].rearrange("p a b -> p (a b)"),
                             start=True, stop=False)

            def start_xT(c, st=None):
                if st is None:
                    st = {}
                    xT = xtp.tile([128, KT, CH], BF16, tag="xT")
                    for g in range(4):
                        nc.sync.dma_start(
                            out=xT[:, 2 * g:2 * g + 2, :],
                            in_=xt_r[:, 2 * g:2 * g + 2, c * CH:(c + 1) * CH])
                    st["xT"] = xT
                return st

            def start_tiles(st):
                st["E"] = actp.tile([128, PAIRS, CH], BF16, tag="E", name="E_t")
                st["sm"] = psP.tile([HC, CH], F32, tag="proj", name="sm")
                st["psik"] = actp.tile([128, PAIRS, F], BF16, tag="psik", name="psik")
                st["v"] = actp.tile([128, PAIRS, HC * (E + 1)], BF16, tag="v", name="v_sb")
                return st

            def emit_q_piece(st, p):
                xT, E_t, sm = st["xT"], st["E"], st["sm"]
                qp = psP.tile([128, CH], F32, tag="proj")
                for kt in range(KT):
                    nc.tensor.matmul(qp[:], wq_sb[:, kt, p * 128:(p + 1) * 128],
                                     xT[:, kt, :], start=(kt == 0), stop=(kt == KT - 1))
                nc.scalar.activation(out=E_t[:, p, :], in_=qp[:], func=AF.Exp)
                nc.tensor.matmul(sm[:], ones_sel[:, p, :], E_t[:, p, :],
                                 start=(p == 0), stop=(p == PAIRS - 1))

            def emit_q_chunk0(st):
                # kt-outer over 4 concurrent PSUM banks: matmuls consume
                # each (wq kt-tile, xT kt-tile) pair as its DMA lands.
                xT, E_t, sm = st["xT"], st["E"], st["sm"]
                qps = [psP.tile([128, CH], F32, tag="proj", name=f"qp{p}")
                       for p in range(PAIRS)]
                for kt in range(KT):
                    for p in range(PAIRS):
                        nc.tensor.matmul(
                            qps[p][:], wq_sb[:, kt, p * 128:(p + 1) * 128],
                            xT[:, kt, :], start=(kt == 0), stop=(kt == KT - 1))
                for p in range(PAIRS):
                    nc.scalar.activation(out=E_t[:, p, :], in_=qps[p][:], func=AF.Exp)
                    nc.tensor.matmul(sm[:], ones_sel[:, p, :], E_t[:, p, :],
                                     start=(p == 0), stop=(p == PAIRS - 1))

            def emit_sm_fin(st):
                rf = smallp.tile([HC, CH], F32, tag="rf")
                nc.vector.reciprocal_approx_fast(out=rf[:], in_=st["sm"][:])
                rbf = smallp.tile([HC, CH], BF16, tag="rbf")
                nc.scalar.activation(out=rbf[:], in_=rf[:], func=AF.Copy)
                st["rbf"] = rbf

            def emit_kv_piece(st, tt):
                xT, psik, v_sb = st["xT"], st["psik"], st["v"]
                kp = psP.tile([128, F], F32, tag="proj")
                for kt in range(KT):
                    nc.tensor.matmul(kp[:], xT[:, kt, tt * 128:(tt + 1) * 128],
                                     wk_sb[:, kt, :], start=(kt == 0), stop=(kt == KT - 1))
                # psi(k) = elu(k)+1 = relu(k) + exp(-relu(-k))
                t1 = tmpp.tile([128, F], BF16, tag="t1")
                nc.scalar.activation(out=t1[:], in_=kp[:], func=AF.Relu, scale=-1.0)
                t2 = tmpp.tile([128, F], BF16, tag="t2")
                nc.scalar.activation(out=t2[:], in_=t1[:], func=AF.Exp, scale=-1.0)
                t3 = tmpp.tile([128, F], BF16, tag="t3")
                nc.scalar.activation(out=t3[:], in_=kp[:], func=AF.Relu)
                nc.vector.tensor_add(psik[:, tt, :], t2[:], t3[:])

                vp = psP.tile([128, F], F32, tag="proj")
                for kt in range(KT):
                    nc.tensor.matmul(vp[:], xT[:, kt, tt * 128:(tt + 1) * 128],
                                     wv_sb[:, kt, :], start=(kt == 0), stop=(kt == KT - 1))
                v3 = v_sb[:, tt, :].rearrange("p (h e1) -> p h e1", e1=E + 1)
                nc.scalar.activation(
                    out=v3[:, :, 0:E],
                    in_=vp[:].rearrange("p (h e) -> p h e", e=E), func=AF.Copy)
                nc.vector.memset(v3[:, :, E:E + 1], 1.0)

            def emit_outproj_half(c, atn_sb, tt, half, st_out):
                if half == 0:
                    st_out["osb"] = outp.tile([128, DIM], BF16, tag="osb",
                                              name="osb")
                osb = st_out["osb"]
                op_ = psP.tile([128, 512], F32, tag="proj")
                for p in range(PAIRS):
                    nc.tensor.matmul(
                        op_[:], atn_sb[:, p, tt * 128:(tt + 1) * 128],
                        wo_sb[:, p, half * 512:(half + 1) * 512],
                        start=(p == 0), stop=(p == PAIRS - 1))
                nc.vector.tensor_copy(out=osb[:, half * 512:(half + 1) * 512],
                                      in_=op_[:])

            def emit_out_dma(c, tt, st_out, half=None):
                osb = st_out["osb"]
                row0 = c * CH + tt * 128
                if half is None:
                    nc.gpsimd.dma_start(out=out[row0:row0 + 128, :], in_=osb[:])
                elif half == 0:
                    nc.gpsimd.dma_start(out=out[row0:row0 + 128, 0:512],
                                        in_=osb[:, 0:512])
                else:
                    nc.sync.dma_start(out=out[row0:row0 + 128, 512:1024],
                                      in_=osb[:, 512:1024])

            def emit_attn(c, st, st_next):
                """Walk chunk c's buckets. Slots: b0-2 carry chunk c's own
                kv tt1-3, b3-6 chunk c+1's q pieces, b7 chunk c+1's kv
                tt0; out-proj pieces at odd buckets."""
                E_t, rbf = st["E"], st["rbf"]
                psik, v_sb = st["psik"], st["v"]
                last = st_next is None

                # q'' = exp(q) * (1/sum exp); 8->128 row broadcast via PE,
                # DVE multiplies straight out of PSUM.
                q2 = actp.tile([128, PAIRS, CH], BF16, tag="q2")
                for p in range(PAIRS):
                    rp = psP.tile([128, CH], F32, tag="proj")
                    nc.tensor.matmul(rp[:], ones_repl[:, p, :], rbf[:],
                                     start=True, stop=True)
                    nc.vector.tensor_tensor(out=q2[:, p, :], in0=E_t[:, p, :],
                                            in1=rp[:], op=OP.mult)

                atn_sb = actp.tile([128, PAIRS, CH], BF16, tag="atnsb")
                st_out = {}
                for j in range(UC):
                    first = (c == 0 and j == 0)
                    if first:
                        cbf = zcbd
                    else:
                        R4 = smallp.tile([128, PAIRS], F32, tag="R4")
                        nc.vector.reciprocal_approx_fast(
                            out=R4[:],
                            in_=C[:, :, E:E + 1].rearrange("p a b -> p (a b)"))
                        cbf = cbfp.tile([128, PAIRS, 128], BF16, tag="cbf")
                        for hh in range(2):
                            r4h = R4[hh * 64:(hh + 1) * 64, :]
                            R4b = bass.AP(
                                tensor=r4h.tensor,
                                offset=r4h.offset,
                                ap=[r4h.ap[0], [r4h.ap[1][0], PAIRS], [0, E]])
                            nc.vector.tensor_tensor(
                                out=cbf[hh * 64:(hh + 1) * 64, :,
                                        hh * 64:hh * 64 + E],
                                in0=C[hh * 64:(hh + 1) * 64, :, 0:E],
                                in1=R4b, op=OP.mult)
                    atn = psA.tile([128, PAIRS, BUCKET], F32, tag="atn")
                    for p in range(PAIRS):
                        # block-diagonal 2-heads-per-matmul: full-width
                        # 128-col weights (FWL) instead of two quadrants
                        nc.tensor.matmul(
                            atn[:, p, :], cbf[:, p, :],
                            q2[:, p, j * 64:(j + 1) * 64],
                            start=True, stop=True)
                    nc.scalar.activation(
                        out=atn_sb[:, :, j * 64:(j + 1) * 64],
                        in_=atn[:], func=AF.Copy)
                    # C += S_j = psi_j^T @ [v_j | 1]
                    tt, r0 = j // 2, (j % 2) * 64
                    v3 = v_sb[:, tt, :].rearrange("p (h e1) -> p h e1", e1=E + 1)
                    for p in range(PAIRS):
                        for hh in range(2):
                            h = 2 * p + hh
                            nc.tensor.matmul(
                                C[hh * 64:(hh + 1) * 64, p, :],
                                psik[r0:r0 + 64, tt, h * E:(h + 1) * E],
                                v3[r0:r0 + 64, h, :],
                                start=False,
                                stop=(c == n_chunks - 1 and j == UC - 1),
                                tile_position=(r0, hh * 64))
                    # fill the stall window behind this serial step
                    if last:
                        # spread own-kv filler deeper into the drain walk
                        if j in (0, 2, 4):
                            emit_kv_piece(st, j // 2 + 1)
                        elif j >= 5:
                            # dummy PE work to cover the exposed serial
                            # chain of the drain walk's thin buckets
                            for _ in range(2):
                                nc.tensor.matmul(
                                    atn[0:64, :, :].rearrange(
                                        "p a b -> p (a b)")[:, 0:192],
                                    ztile[:, 0:64],
                                    zcbf[0:64, :, :].rearrange(
                                        "p a b -> p (a b)")[:, 0:192],
                                    start=True, stop=True)
                    elif j < 3:
                        emit_kv_piece(st, j + 1)
                    elif not last:
                        if j == 3:
                            start_tiles(st_next)
                        if j < 7:
                            emit_q_piece(st_next, j - 3)
                            if j == 6:
                                emit_sm_fin(st_next)
                        else:
                            emit_kv_piece(st_next, 0)
                    if not last:
                        if j % 2 == 1:
                            tt_o = (j - 1) // 2
                            emit_outproj_half(c, atn_sb, tt_o, 0, st_out)
                            emit_outproj_half(c, atn_sb, tt_o, 1, st_out)
                            emit_out_dma(c, tt_o, st_out)
                    else:
                        if j >= 1:
                            tt_o, half = (j - 1) // 2, (j - 1) % 2
                            emit_outproj_half(c, atn_sb, tt_o, half, st_out)
                            emit_out_dma(c, tt_o, st_out, half=half)
                if last:
                    emit_outproj_half(c, atn_sb, 3, 1, st_out)
                    emit_out_dma(c, 3, st_out, half=1)

            st = start_xT(0, st={"xT": xT0})
            start_tiles(st)
            emit_q_chunk0(st)
            emit_sm_fin(st)
            emit_kv_piece(st, 0)
            for c in range(n_chunks):
                st_next = start_xT(c + 1) if c + 1 < n_chunks else None
                emit_attn(c, st, st_next)
                st = st_next

    nc.finalize()
    return nc


def _orp():
    m = np.zeros((HC, PAIRS, 128), dtype=ml_dtypes.bfloat16)
    for p in range(PAIRS):
        for hh in range(2):
            m[2 * p + hh, p, hh * 64:(hh + 1) * 64] = 1
    return m.reshape(HC, PAIRS * 128)


def _pmajor_w(W):
    # [DIM, F] -> [128, KT*F]: row d = kt*128 + p goes to [p, kt, :]
    return np.ascontiguousarray(
        W.reshape(KT, 128, F).transpose(1, 0, 2).reshape(128, KT * F))


def make_in_maps(inputs):
    bf = ml_dtypes.bfloat16
    x = np.asarray(inputs["x"])
    Wq, Wk, Wv, Wo = (np.asarray(inputs[n]) for n in ("Wq", "Wk", "Wv", "Wo"))
    in_maps = []
    for c in range(8):
        b, g = c // 2, c % 2
        sl = slice(g * F, (g + 1) * F)
        xtb = x[b].T.astype(bf)  # [DIM, T]
        in_maps.append({
            "xt": np.ascontiguousarray(
                xtb.reshape(KT, 128, T).transpose(1, 0, 2)),
            "wq": _pmajor_w(Wq[:, sl].astype(bf)),
            "wk": _pmajor_w(Wk[:, sl].astype(bf)),
            "wv": _pmajor_w(Wv[:, sl].astype(bf)),
            "wo": np.ascontiguousarray(
                Wo[sl, :].astype(bf).reshape(PAIRS, 128, DIM)
                .transpose(1, 0, 2).reshape(128, PAIRS * DIM)),
            "orp": _orp(),
        })
    return in_maps


def kernel(x, Wq, Wk, Wv, Wo, bo):
    from concourse.bass_utils import run_bass_kernel_spmd

    if "nc" not in _NC_CACHE:
        _NC_CACHE["nc"] = build_nc()
    nc = _NC_CACHE["nc"]

    in_maps = make_in_maps({"x": x, "Wq": Wq, "Wk": Wk, "Wv": Wv, "Wo": Wo})
    res = run_bass_kernel_spmd(nc, in_maps, core_ids=list(range(8)))
    outs = [res.results[c]["out"].astype(np.float32) for c in range(8)]
    full = np.stack([outs[2 * b] + outs[2 * b + 1] for b in range(B)], axis=0)
    return (full + np.asarray(bo)[None, None, :]).astype(np.float32)


# revision 23
# speedup vs baseline: 1.0082x; 1.0082x over previous
"""Trainium2 Bass kernel v5 for bucketed causal linear self-attention.

Model (B=4, T=4096, DIM=1024, H=16 heads, E=64, BUCKET=64):
  q,k,v = x@Wq, x@Wk, x@Wv ; q softmaxed over head-dim, k -> elu(k)+1
  per-bucket context C_u = cumsum_u(k_bu^T v_bu), normalized by cumsum of
  key-sums, shifted one bucket; attn_bu = q_bu @ C_{u-1}; out = attn@Wo + bo.

Sharding over 8 cores: core c -> batch c//2, head-group c%2 (8 heads = 512
feats). Host transposes x (so no on-device DMA transposes), sums the two
partial outputs per batch and adds bo.

v5 vs v2:
  - steady-state slot schedule: walk c's bucket slots carry chunk c's OWN
    kv tt1-3 (b0-2), chunk c+1's q pieces (b3-6, + sm fin), and chunk
    c+1's kv tt0 (b7). This shrinks chunk 0's serial pre-walk phase from
    q+all-kv to q+kv(tt0) and gives the last walk real filler.
  - prologue DMAs ordered by first use (wq+xT0 interleaved, then wk, wv,
    wo halves, orp) across the three DMA-capable queues.
  - chunk-0 q-projection runs kt-outer over 4 concurrent PSUM banks so
    matmuls start as each (wq kt, xT kt) pair lands.
  - proj-tile PSUM allocs happen at slot b3 (after the q2 multiplies
    consumed the broadcast banks) to avoid bank-recycle stalls.
  - q2 = E_t * rp with rp read straight from PSUM by the DVE.
  - cbf snapshot is one DVE op per bucket.
  - last chunk interleaves out-proj half-pieces at every bucket; final
    stores split per half to drain earlier.
"""

import sys
import numpy as np
import ml_dtypes

sys.path.insert(0, "/opt/trn_rl_repo")

B, T, DIM, H, BUCKET = 4, 4096, 1024, 16, 64
E = 64           # head dim
HC = 8           # heads per core
F = HC * E       # per-core feature width = 512
CH = 512         # tokens per chunk
UC = CH // BUCKET  # buckets per chunk = 8
PAIRS = HC // 2  # head pairs = 4
KT = DIM // 128  # contraction tiles = 8
NCH = T // CH    # chunks = 8

_NC_CACHE = {}


def build_nc(n_chunks=NCH):
    import concourse.bass as bass
    import concourse.mybir as mybir
    from concourse import bacc
    from concourse.tile import TileContext

    BF16 = mybir.dt.bfloat16
    F32 = mybir.dt.float32
    AF = mybir.ActivationFunctionType
    OP = mybir.AluOpType

    Tt = n_chunks * CH

    nc = bacc.Bacc("TRN2", target_bir_lowering=False, debug=False, num_devices=8)
    # weights arrive host-prearranged partition-major so loads are contiguous
    xt = nc.dram_tensor("xt", [128, KT, Tt], BF16, kind="ExternalInput").ap()
    wq = nc.dram_tensor("wq", [128, KT * F], BF16, kind="ExternalInput").ap()
    wk = nc.dram_tensor("wk", [128, KT * F], BF16, kind="ExternalInput").ap()
    wv = nc.dram_tensor("wv", [128, KT * F], BF16, kind="ExternalInput").ap()
    wo = nc.dram_tensor("wo", [128, PAIRS * DIM], BF16, kind="ExternalInput").ap()
    orp = nc.dram_tensor("orp", [HC, PAIRS * 128], BF16, kind="ExternalInput").ap()
    out = nc.dram_tensor("out", [Tt, DIM], BF16, kind="ExternalOutput").ap()

    xt_r = xt

    with TileContext(nc) as tc:
        with tc.tile_pool(name="const", bufs=1) as constp, \
             tc.tile_pool(name="xt", bufs=2) as xtp, \
             tc.tile_pool(name="act", bufs=2) as actp, \
             tc.tile_pool(name="tmp", bufs=3) as tmpp, \
             tc.tile_pool(name="small", bufs=6) as smallp, \
             tc.tile_pool(name="cbfp", bufs=6) as cbfp, \
             tc.tile_pool(name="outp", bufs=3) as outp, \
             tc.tile_pool(name="ps_proj", bufs=5, space="PSUM") as psP, \
             tc.tile_pool(name="ps_atn", bufs=2, space="PSUM") as psA, \
             tc.tile_pool(name="ps_c", bufs=1, space="PSUM") as psC:

            # ---- resident weights + chunk-0 xT: issue in order of first
            # use so the 16 HW DMA engines serve critical bytes first.
            wq_sb = constp.tile([128, KT, F], BF16, tag="wq")
            wk_sb = constp.tile([128, KT, F], BF16, tag="wk")
            wv_sb = constp.tile([128, KT, F], BF16, tag="wv")
            wo_sb = constp.tile([128, PAIRS, DIM], BF16, tag="wo")
            wq_r = wq.rearrange("p (kt f) -> p kt f", f=F)
            wk_r = wk.rearrange("p (kt f) -> p kt f", f=F)
            wv_r = wv.rearrange("p (kt f) -> p kt f", f=F)
            wo_r = wo.rearrange("p (ft n) -> p ft n", n=DIM)

            xT0 = xtp.tile([128, KT, CH], BF16, tag="xT")
            for kt in range(KT):
                qeng = nc.scalar if kt % 2 == 0 else nc.gpsimd
                qeng.dma_start(out=wq_sb[:, kt, :], in_=wq_r[:, kt, :])
                if kt % 2 == 0:
                    g = kt // 2
                    nc.sync.dma_start(out=xT0[:, 2 * g:2 * g + 2, :],
                                      in_=xt_r[:, 2 * g:2 * g + 2, 0:CH])
            for g in range(4):
                eng = nc.scalar if g % 2 == 0 else nc.gpsimd
                eng.dma_start(out=wk_sb[:, 2 * g:2 * g + 2, :],
                              in_=wk_r[:, 2 * g:2 * g + 2, :])
            for g in range(4):
                eng = nc.scalar if g % 2 == 0 else nc.gpsimd
                eng.dma_start(out=wv_sb[:, 2 * g:2 * g + 2, :],
                              in_=wv_r[:, 2 * g:2 * g + 2, :])
            # ones_repl[:, p, :]: [8,128] stationary; row 2p+hh is 1 on cols hh*64..
            ones_repl = constp.tile([HC, PAIRS, 128], BF16, tag="ones_repl")
            nc.sync.dma_start(
                out=ones_repl[:],
                in_=orp.rearrange("h (p c) -> h p c", c=128))
            for p in range(PAIRS):
                eng = nc.scalar if p % 2 == 0 else nc.gpsimd
                eng.dma_start(out=wo_sb[:, p, :], in_=wo_r[:, p, :])

            # ones_sel[:, p, :]: [128,8] stationary; col 2p+hh is 1 on rows hh*64..
            ones_sel = constp.tile([128, PAIRS, HC], BF16, tag="ones_sel")
            nc.vector.memset(ones_sel[:], 0.0)
            for p in range(PAIRS):
                for hh in range(2):
                    nc.vector.memset(
                        ones_sel[hh * 64:(hh + 1) * 64, p, 2 * p + hh:2 * p + hh + 1], 1.0)
            # zero context for the very first bucket (blindspot); zcbd is the
            # block-diagonal [128,128]-per-pair form the attn matmuls consume
            zcbf = constp.tile([128, PAIRS, E + 1], BF16, tag="zcbf")
            nc.vector.memset(zcbf[:], 0.0)
            ztile = constp.tile([64, 128], BF16, tag="ztile")
            nc.vector.memset(ztile[:], 0.0)
            zcbd = constp.tile([128, PAIRS, 128], BF16, tag="zcbd")
            nc.vector.memset(zcbd[:], 0.0)
            # pre-zero all cbf pool buffers once: walk uses only rewrite the
            # two diagonal blocks, so the off-diagonal zeros persist.
            for zi in range(6):
                zb = cbfp.tile([128, PAIRS, 128], BF16, tag="cbf", name=f"cbz{zi}")
                nc.vector.memset(zb[:], 0.0)

            # running context+ksum per head pair, PSUM-resident: [2*64 d, p, 64 e + 1 ks]
            C = psC.tile([128, PAIRS, E + 1], F32, tag="C")
            # Zero-init the whole C region with ONE start=True matmul: the
            # bank-wide has_written clear must happen exactly once, before
            # any S write, and the WAW overlap with every later quadrant
            # write pins this matmul first in the schedule.
            nc.tensor.matmul(C[:].rearrange("p a b -> p (a b)"), ztile[:],
                             zcbf[0:64, :, :].rearrange("p a b -> p (a b)"),
                             start=True, stop=False)

            def start_xT(c, st=None):
                if st is None:
                    st = {}
                    xT = xtp.tile([128, KT, CH], BF16, tag="xT")
                    for g in range(4):
                        nc.sync.dma_start(
                            out=xT[:, 2 * g:2 * g + 2, :],
                            in_=xt_r[:, 2 * g:2 * g + 2, c * CH:(c + 1) * CH])
                    st["xT"] = xT
                return st

            def start_tiles(st):
                st["E"] = actp.tile([128, PAIRS, CH], BF16, tag="E", name="E_t")
                st["sm"] = psP.tile([HC, CH], F32, tag="proj", name="sm")
                st["psik"] = actp.tile([128, PAIRS, F], BF16, tag="psik", name="psik")
                st["v"] = actp.tile([128, PAIRS, HC * (E + 1)], BF16, tag="v", name="v_sb")
                return st

            def emit_q_piece(st, p):
                xT, E_t, sm = st["xT"], st["E"], st["sm"]
                qp = psP.tile([128, CH], F32, tag="proj")
                for kt in range(KT):
                    nc.tensor.matmul(qp[:], wq_sb[:, kt, p * 128:(p + 1) * 128],
                                     xT[:, kt, :], start=(kt == 0), stop=(kt == KT - 1))
                nc.scalar.activation(out=E_t[:, p, :], in_=qp[:], func=AF.Exp)
                nc.tensor.matmul(sm[:], ones_sel[:, p, :], E_t[:, p, :],
                                 start=(p == 0), stop=(p == PAIRS - 1))

            def emit_q_chunk0(st):
                # kt-outer over 4 concurrent PSUM banks: matmuls consume
                # each (wq kt-tile, xT kt-tile) pair as its DMA lands.
                xT, E_t, sm = st["xT"], st["E"], st["sm"]
                qps = [psP.tile([128, CH], F32, tag="proj", name=f"qp{p}")
                       for p in range(PAIRS)]
                for kt in range(KT):
                    for p in range(PAIRS):
                        nc.tensor.matmul(
                            qps[p][:], wq_sb[:, kt, p * 128:(p + 1) * 128],
                            xT[:, kt, :], start=(kt == 0), stop=(kt == KT - 1))
                for p in range(PAIRS):
                    nc.scalar.activation(out=E_t[:, p, :], in_=qps[p][:], func=AF.Exp)
                    nc.tensor.matmul(sm[:], ones_sel[:, p, :], E_t[:, p, :],
                                     start=(p == 0), stop=(p == PAIRS - 1))

            def emit_sm_fin(st):
                rf = smallp.tile([HC, CH], F32, tag="rf")
                nc.vector.reciprocal_approx_fast(out=rf[:], in_=st["sm"][:])
                rbf = smallp.tile([HC, CH], BF16, tag="rbf")
                nc.scalar.activation(out=rbf[:], in_=rf[:], func=AF.Copy)
                st["rbf"] = rbf

            def emit_kv_piece(st, tt):
                xT, psik, v_sb = st["xT"], st["psik"], st["v"]
                kp = psP.tile([128, F], F32, tag="proj")
                for kt in range(KT):
                    nc.tensor.matmul(kp[:], xT[:, kt, tt * 128:(tt + 1) * 128],
                                     wk_sb[:, kt, :], start=(kt == 0), stop=(kt == KT - 1))
                # psi(k) = elu(k)+1 = relu(k) + exp(-relu(-k))
                t1 = tmpp.tile([128, F], BF16, tag="t1")
                nc.scalar.activation(out=t1[:], in_=kp[:], func=AF.Relu, scale=-1.0)
                t2 = tmpp.tile([128, F], BF16, tag="t2")
                nc.scalar.activation(out=t2[:], in_=t1[:], func=AF.Exp, scale=-1.0)
                t3 = tmpp.tile([128, F], BF16, tag="t3")
                nc.scalar.activation(out=t3[:], in_=kp[:], func=AF.Relu)
                nc.vector.tensor_add(psik[:, tt, :], t2[:], t3[:])

                vp = psP.tile([128, F], F32, tag="proj")
                for kt in range(KT):
                    nc.tensor.matmul(vp[:], xT[:, kt, tt * 128:(tt + 1) * 128],
                                     wv_sb[:, kt, :], start=(kt == 0), stop=(kt == KT - 1))
                v3 = v_sb[:, tt, :].rearrange("p (h e1) -> p h e1", e1=E + 1)
                nc.scalar.activation(
                    out=v3[:, :, 0:E],
                    in_=vp[:].rearrange("p (h e) -> p h e", e=E), func=AF.Copy)
                nc.vector.memset(v3[:, :, E:E + 1], 1.0)

            def emit_outproj_half(c, atn_sb, tt, half, st_out):
                if half == 0:
                    st_out["osb"] = outp.tile([128, DIM], BF16, tag="osb",
                                              name="osb")
                osb = st_out["osb"]
                op_ = psP.tile([128, 512], F32, tag="proj")
                for p in range(PAIRS):
                    nc.tensor.matmul(
                        op_[:], atn_sb[:, p, tt * 128:(tt + 1) * 128],
                        wo_sb[:, p, half * 512:(half + 1) * 512],
                        start=(p == 0), stop=(p == PAIRS - 1))
                nc.vector.tensor_copy(out=osb[:, half * 512:(half + 1) * 512],
                                      in_=op_[:])

            def emit_out_dma(c, tt, st_out, half=None):
                osb = st_out["osb"]
                row0 = c * CH + tt * 128
                if half is None:
                    nc.gpsimd.dma_start(out=out[row0:row0 + 128, :], in_=osb[:])
                elif half == 0:
                    nc.gpsimd.dma_start(out=out[row0:row0 + 128, 0:512],
                                        in_=osb[:, 0:512])
                else:
                    nc.sync.dma_start(out=out[row0:row0 + 128, 512:1024],
                                      in_=osb[:, 512:1024])

            def emit_attn(c, st, st_next):
                """Walk chunk c's buckets. Slots: b0-2 carry chunk c's own
                kv tt1-3, b3-6 chunk c+1's q pieces, b7 chunk c+1's kv
                tt0; out-proj pieces at odd buckets."""
                E_t, rbf = st["E"], st["rbf"]
                psik, v_sb = st["psik"], st["v"]
                last = st_next is None

                # q'' = exp(q) * (1/sum exp); 8->128 row broadcast via PE,
                # DVE multiplies straight out of PSUM.
                q2 = actp.tile([128, PAIRS, CH], BF16, tag="q2")
                for p in range(PAIRS):
                    rp = psP.tile([128, CH], F32, tag="proj")
                    nc.tensor.matmul(rp[:], ones_repl[:, p, :], rbf[:],
                                     start=True, stop=True)
                    nc.vector.tensor_tensor(out=q2[:, p, :], in0=E_t[:, p, :],
                                            in1=rp[:], op=OP.mult)

                atn_sb = actp.tile([128, PAIRS, CH], BF16, tag="atnsb")
                st_out = {}
                for j in range(UC):
                    first = (c == 0 and j == 0)
                    if first:
                        cbf = zcbd
                    else:
                        R4 = smallp.tile([128, PAIRS], F32, tag="R4")
                        nc.vector.reciprocal_approx_fast(
                            out=R4[:],
                            in_=C[:, :, E:E + 1].rearrange("p a b -> p (a b)"))
                        cbf = cbfp.tile([128, PAIRS, 128], BF16, tag="cbf")
                        for hh in range(2):
                            r4h = R4[hh * 64:(hh + 1) * 64, :]
                            R4b = bass.AP(
                                tensor=r4h.tensor,
                                offset=r4h.offset,
                                ap=[r4h.ap[0], [r4h.ap[1][0], PAIRS], [0, E]])
                            nc.vector.tensor_tensor(
                                out=cbf[hh * 64:(hh + 1) * 64, :,
                                        hh * 64:hh * 64 + E],
                                in0=C[hh * 64:(hh + 1) * 64, :, 0:E],
                                in1=R4b, op=OP.mult)
                    atn = psA.tile([128, PAIRS, BUCKET], F32, tag="atn")
                    for p in range(PAIRS):
                        # block-diagonal 2-heads-per-matmul: full-width
                        # 128-col weights (FWL) instead of two quadrants
                        nc.tensor.matmul(
                            atn[:, p, :], cbf[:, p, :],
                            q2[:, p, j * 64:(j + 1) * 64],
                            start=True, stop=True)
                    nc.scalar.activation(
                        out=atn_sb[:, :, j * 64:(j + 1) * 64],
                        in_=atn[:], func=AF.Copy)
                    # C += S_j = psi_j^T @ [v_j | 1]
                    tt, r0 = j // 2, (j % 2) * 64
                    v3 = v_sb[:, tt, :].rearrange("p (h e1) -> p h e1", e1=E + 1)
                    for p in range(PAIRS):
                        for hh in range(2):
                            h = 2 * p + hh
                            nc.tensor.matmul(
                                C[hh * 64:(hh + 1) * 64, p, :],
                                psik[r0:r0 + 64, tt, h * E:(h + 1) * E],
                                v3[r0:r0 + 64, h, :],
                                start=False,
                                stop=(c == n_chunks - 1 and j == UC - 1),
                                tile_position=(r0, hh * 64))
                    # fill the stall window behind this serial step
                    if j < 3:
                        emit_kv_piece(st, j + 1)
                    elif not last:
                        if j == 3:
                            start_tiles(st_next)
                        if j < 7:
                            emit_q_piece(st_next, j - 3)
                            if j == 6:
                                emit_sm_fin(st_next)
                        else:
                            emit_kv_piece(st_next, 0)
                    if not last:
                        if j % 2 == 1:
                            tt_o = (j - 1) // 2
                            emit_outproj_half(c, atn_sb, tt_o, 0, st_out)
                            emit_outproj_half(c, atn_sb, tt_o, 1, st_out)
                            emit_out_dma(c, tt_o, st_out)
                    else:
                        if j >= 1:
                            tt_o, half = (j - 1) // 2, (j - 1) % 2
                            emit_outproj_half(c, atn_sb, tt_o, half, st_out)
                            emit_out_dma(c, tt_o, st_out, half=half)
                if last:
                    emit_outproj_half(c, atn_sb, 3, 1, st_out)
                    emit_out_dma(c, 3, st_out, half=1)

            st = start_xT(0, st={"xT": xT0})
            start_tiles(st)
            emit_q_chunk0(st)
            emit_sm_fin(st)
            emit_kv_piece(st, 0)
            for c in range(n_chunks):
                st_next = start_xT(c + 1) if c + 1 < n_chunks else None
                emit_attn(c, st, st_next)
                st = st_next

    nc.finalize()
    return nc


def _orp():
    m = np.zeros((HC, PAIRS, 128), dtype=ml_dtypes.bfloat16)
    for p in range(PAIRS):
        for hh in range(2):
            m[2 * p + hh, p, hh * 64:(hh + 1) * 64] = 1
    return m.reshape(HC, PAIRS * 128)


def _pmajor_w(W):
    # [DIM, F] -> [128, KT*F]: row d = kt*128 + p goes to [p, kt, :]
    return np.ascontiguousarray(
        W.reshape(KT, 128, F).transpose(1, 0, 2).reshape(128, KT * F))


def make_in_maps(inputs):
    bf = ml_dtypes.bfloat16
    x = np.asarray(inputs["x"])
    Wq, Wk, Wv, Wo = (np.asarray(inputs[n]) for n in ("Wq", "Wk", "Wv", "Wo"))
    in_maps = []
    for c in range(8):
        b, g = c // 2, c % 2
        sl = slice(g * F, (g + 1) * F)
        xtb = x[b].T.astype(bf)  # [DIM, T]
        in_maps.append({
            "xt": np.ascontiguousarray(
                xtb.reshape(KT, 128, T).transpose(1, 0, 2)),
            "wq": _pmajor_w(Wq[:, sl].astype(bf)),
            "wk": _pmajor_w(Wk[:, sl].astype(bf)),
            "wv": _pmajor_w(Wv[:, sl].astype(bf)),
            "wo": np.ascontiguousarray(
                Wo[sl, :].astype(bf).reshape(PAIRS, 128, DIM)
                .transpose(1, 0, 2).reshape(128, PAIRS * DIM)),
            "orp": _orp(),
        })
    return in_maps


def kernel(x, Wq, Wk, Wv, Wo, bo):
    from concourse.bass_utils import run_bass_kernel_spmd

    if "nc" not in _NC_CACHE:
        _NC_CACHE["nc"] = build_nc()
    nc = _NC_CACHE["nc"]

    in_maps = make_in_maps({"x": x, "Wq": Wq, "Wk": Wk, "Wv": Wv, "Wo": Wo})
    res = run_bass_kernel_spmd(nc, in_maps, core_ids=list(range(8)))
    outs = [res.results[c]["out"].astype(np.float32) for c in range(8)]
    full = np.stack([outs[2 * b] + outs[2 * b + 1] for b in range(B)], axis=0)
    return (full + np.asarray(bo)[None, None, :]).astype(np.float32)
